# revision 5
# baseline (speedup 1.0000x reference)
"""Multi-head attention (B=4, S=2048, D=1024, H=16) on 8 trn2 NeuronCores.

Sharding: batch x head-group. Core c handles batch b=c//2 and head group
g=c%2 (8 of 16 heads = a 512-column slice of the Q/K/V projections and a
512-row slice of the O projection). The host passes transposed bf16
operands (X^T, W^T, mask^T) so the device does no input casts/transposes;
the host sums the two partial O-projection outputs per batch and adds bo.

Per-core pipeline (local heads h=0..7):
  - Q/K proj -> qt/kt [128(dk, pair-packed), hp, 2048] bf16: psum eviction
    fused with per-partition bias add (DVE tensor_scalar).
  - V proj -> v [128(s), st, h, 65] bf16 (col 64 = ones, the softmax
    denominator trick), fused broadcast bias add (Pool).
  - Attention per (h, r-half), st-loop software-pipelined so the PE order
    is sc(st+1) before A@V(st) and never waits on the ACT exp chain:
      scores St[s,r]: psum [128,1024], 2 matmuls (K=64)
      Pexp = exp(0.125*St) bf16 (ACT -- sole user of the ACT queue, the
        critical resource at ~267us model time)
      Pexp *= mask^T slice (DVE, 2x bf16)
      A@V flipped: x[r, d|den] += Pexp_block^T @ [V_h | 1] (lhsT = Pexp
        block, N=65, full 128 output partitions)
    End of r-half: one Pool copy psum->SBUF bf16 frees the psum
    immediately; reciprocal + per-rt tensor_scalar normalize run off the
    critical path -> x [128(r), rt, 512] bf16.
  - O proj per t-tile: PE-transpose x, matmul vs woT, Pool evict, DMA out
    (partial f32; host adds the pair + bo).
All loads are deadline-ordered and later head-pairs' projections are
emitted as background units inside earlier heads' attention loops (loads
two slots ahead of their matmuls) so DMA prefetches and the PE never
drains the ACT exp stream. O-proj t-tiles 0-7 overlap the last r-half.
"""

import numpy as np

import concourse.bass as bass
import concourse.bacc as bacc
import concourse.mybir as mybir
import concourse.tile as tile
from concourse.masks import make_identity

F32 = mybir.dt.float32
BF16 = mybir.dt.bfloat16

B, S, D, H, DK = 4, 2048, 1024, 16, 64
P = 128
NCORES = 8
DH = D // 2          # 512 head dims per core (8 heads)
H8 = H // 2          # heads per core
NHP = 4              # head pairs per core (qt/kt partition-packed)
ST = S // P          # 16 s-tiles
RT = S // P          # 16 r-tiles
RC = 512             # projection free-dim chunk
NKT = D // P         # 8 contraction tiles (d_in)
OKT = DH // P        # 4 contraction tiles for O proj
RH = S // 2          # 1024 r-half


def build_nc():
    nc = bacc.Bacc("TRN2", target_bir_lowering=False, debug=False,
                   num_devices=NCORES)

    xqT = nc.declare_dram_parameter("xqT", [D, S], BF16, isOutput=False)
    xkT = nc.declare_dram_parameter("xkT", [D, S], BF16, isOutput=False)
    xvT = nc.declare_dram_parameter("xvT", [D, S], BF16, isOutput=False)
    wqT = nc.declare_dram_parameter("wqT", [D, DH], BF16, isOutput=False)
    wkT = nc.declare_dram_parameter("wkT", [D, DH], BF16, isOutput=False)
    wvT = nc.declare_dram_parameter("wvT", [D, DH], BF16, isOutput=False)
    woT = nc.declare_dram_parameter("woT", [DH, D], BF16, isOutput=False)
    bq = nc.declare_dram_parameter("bq", [DH], F32, isOutput=False)
    bk = nc.declare_dram_parameter("bk", [DH], F32, isOutput=False)
    bv = nc.declare_dram_parameter("bv", [DH], F32, isOutput=False)
    mskT = nc.declare_dram_parameter("mskT", [S, S], BF16, isOutput=False)
    out = nc.declare_dram_parameter("out", [S, D], F32, isOutput=True)

    xqT_v = xqT.ap().rearrange("(k p) t -> p k t", p=P)
    xkT_v = xkT.ap().rearrange("(k p) t -> p k t", p=P)
    xvT_v = xvT.ap().rearrange("(k p) t -> p k t", p=P)
    mt_view = mskT.ap().rearrange("(st p) r -> p st r", p=P)

    with tile.TileContext(nc) as tc:
        with (
            tc.tile_pool(name="const", bufs=1) as const,
            tc.tile_pool(name="persist", bufs=1) as persist,
            tc.tile_pool(name="xring", bufs=2) as xring,
            tc.tile_pool(name="vring", bufs=3) as vring,
            tc.tile_pool(name="pexp", bufs=2) as pexpp,
            tc.tile_pool(name="stage", bufs=2) as stpool,
            tc.tile_pool(name="small", bufs=4) as small,
        ):
            identity = const.tile([P, P], BF16)
            make_identity(nc, identity)

            ps_scope_sc = tc.tile_pool(name="sc_ps", bufs=3, space="PSUM")
            scpool = ps_scope_sc.__enter__()
            ps_scope_xt = tc.tile_pool(name="xt_ps", bufs=1, space="PSUM")
            xtpool = ps_scope_xt.__enter__()

            bq_sb = const.tile([P, NHP], F32)
            bk_sb = const.tile([P, NHP], F32)
            bv_bc = const.tile([P, DH], F32)
            wo_sb = const.tile([P, OKT, D], BF16)
            mt_sb = persist.tile([P, ST, S], BF16)   # mask^T [s, st, r]
            qt_sb = persist.tile([P, NHP, S], BF16)  # [dk-pair, hp, t]
            kt_sb = persist.tile([P, NHP, S], BF16)  # [dk-pair, hp, s]
            v_sb = persist.tile([P, ST, H8, DK + 1], BF16)  # [s, st, h, d|1]
            x_sb = persist.tile([P, RT, DH], BF16)   # [r, rt, d-all-heads]

            w_scope = tc.tile_pool(name="wqkv", bufs=1)
            wpool = w_scope.__enter__()
            wq_sb = wpool.tile([P, NKT, DH], BF16, tag="wq")
            wk_sb = wpool.tile([P, NKT, DH], BF16, tag="wk")
            wv_sb = wpool.tile([P, NKT, DH], BF16, tag="wv")

            def load_col_bias(dst, src):
                nc.sync.dma_start(out=dst,
                                  in_=src.ap().rearrange("(t p) -> p t", p=P))

            def load_mask_g2(g2):
                nc.sync.dma_start(out=mt_sb[:, g2 * 2:(g2 + 1) * 2, :],
                                  in_=mt_view[:, g2 * 2:(g2 + 1) * 2, :])

            def qk_load(xv_, tc_, tag):
                xc = xring.tile([P, NKT, RC], BF16, tag=tag, name=tag)
                nc.sync.dma_start(out=xc, in_=xv_[:, :, tc_ * RC:(tc_ + 1) * RC])
                return xc

            def qk_mm(hp, w_sb, b_sb, xc, dst, tc_):
                ps = scpool.tile([P, RC], F32, tag="sc", name="pj_ps")
                for kt in range(NKT):
                    nc.tensor.matmul(
                        ps,
                        w_sb[:, kt, hp * P:(hp + 1) * P],
                        xc[:, kt, :],
                        start=(kt == 0), stop=(kt == NKT - 1))
                nc.vector.tensor_scalar_add(
                    dst[:, hp, tc_ * RC:(tc_ + 1) * RC], ps, b_sb[:, hp:hp + 1])

            def v_load(st):
                xc = vring.tile([P, NKT, P], BF16, tag="xv", name="xv")
                nc.sync.dma_start(out=xc,
                                  in_=xvT_v[:, :, st * P:(st + 1) * P])
                return xc

            def v_mm(st, xc):
                ps = scpool.tile([P, RC], F32, tag="sc", name="pj_ps")
                for kt in range(NKT):
                    nc.tensor.matmul(
                        ps,
                        xc[:, kt, :],
                        wv_sb[:, kt, :],
                        start=(kt == 0), stop=(kt == NKT - 1))
                nc.vector.tensor_add(
                    v_sb[:, st, :, 0:DK],
                    ps.rearrange("p (h d) -> p h d", h=H8),
                    bv_bc.rearrange("p (h d) -> p h d", h=H8))

            sc_tiles = {}

            def emit_sc(h, rh, st):
                hp, hoff = h // 2, (h % 2) * DK
                r0 = rh * RH
                sc = scpool.tile([P, RH], F32, tag="sc", name="sc_ps")
                for rc_ in range(RH // RC):
                    nc.tensor.matmul(
                        sc[:, rc_ * RC:(rc_ + 1) * RC],
                        kt_sb[hoff:hoff + DK, hp, st * P:(st + 1) * P],
                        qt_sb[hoff:hoff + DK, hp,
                              r0 + rc_ * RC:r0 + (rc_ + 1) * RC],
                        start=True, stop=True)
                sc_tiles[(h, rh, st)] = sc

            def attn_rhalf_body(idx, rhalves):
                h, rh, units, bg, bg_sts = rhalves[idx]
                hoff = (h % 2) * DK
                r0 = rh * RH
                xt = xtpool.tile([P, RT // 2, P], F32, tag="xt", name="xt_ps")
                nc.vector.memset(xt[:, :, 0:DK + 1], 0.0)
                for st in range(ST):
                    pexp = pexpp.tile([P, RH], BF16, tag="pexp", name="pexp")
                    nc.scalar.activation(
                        pexp, sc_tiles.pop((h, rh, st)),
                        mybir.ActivationFunctionType.Exp, scale=0.125)
                    nc.vector.tensor_mul(pexp, pexp, mt_sb[:, st, r0:r0 + RH])
                    if units is not None:
                        for u in units.get(st, []):
                            u()
                    if bg is not None and st in bg_sts:
                        bg.pop(0)()
                    # two-tile score lookahead, crossing rhalf boundaries
                    n_idx, n_st = idx, st + 2
                    if n_st >= ST:
                        n_idx, n_st = idx + 1, n_st - ST
                    if n_idx < len(rhalves):
                        emit_sc(rhalves[n_idx][0], rhalves[n_idx][1], n_st)
                    for rt in range(RT // 2):
                        # psum zero-regions are 2048B = 4 rt slices, so the
                        # usual start=True protocol would wipe sibling
                        # slices. Instead the psum is zeroed explicitly by
                        # the DVE memset above and every matmul accumulates
                        # (start=False), which is also insensitive to stale
                        # pending-zero state left by earlier kernels.
                        nc.tensor.matmul(
                            xt[:, rt, 0:DK + 1],
                            pexp[:, rt * P:(rt + 1) * P],
                            v_sb[:, st, h, :],
                            start=False, stop=(st == ST - 1),
                            skip_group_check=True)
                # stage to SBUF (frees psum), then normalize off-path
                stg = stpool.tile([P, RT // 2, DK + 1], BF16, tag="stg")
                nc.vector.tensor_copy(out=stg, in_=xt[:, :, 0:DK + 1])
                recip = small.tile([P, RT // 2], F32, tag="recip")
                nc.vector.reciprocal(recip, stg[:, :, DK])
                for rt in range(RT // 2):
                    rt_g = rh * (RT // 2) + rt
                    nc.vector.tensor_scalar_mul(
                        x_sb[:, rt_g, h * DK:(h + 1) * DK],
                        stg[:, rt, 0:DK], recip[:, rt:rt + 1])

            xT_tiles = {}

            def o_tp(t, pool, ev_eng):
                ps_t = pool.tile([P, RC], BF16, tag="sc", name="tp_ps")
                for kt in range(OKT):
                    nc.tensor.transpose(ps_t[:, kt * P:(kt + 1) * P],
                                        x_sb[:, t, kt * P:(kt + 1) * P],
                                        identity)
                xT_rt = ostage.tile([P, OKT, P], BF16, tag="xT", name="xT_rt")
                ev_eng(out=xT_rt, in_=ps_t.rearrange("p (k q) -> p k q", k=OKT))
                xT_tiles[t] = xT_rt

            def o_mm(t, pool, ev_eng):
                xT_rt = xT_tiles.pop(t)
                for oc in range(D // RC):
                    ps = pool.tile([P, RC], F32, tag="sc", name="o_ps")
                    for kt in range(OKT):
                        nc.tensor.matmul(
                            ps,
                            xT_rt[:, kt, :],
                            wo_sb[:, kt, oc * RC:(oc + 1) * RC],
                            start=(kt == 0), stop=(kt == OKT - 1))
                    ev = ostage.tile([P, RC], F32, tag="o_ev", name="o_ev")
                    ev_eng(out=ev, in_=ps)
                    nc.sync.dma_start(
                        out=out[t * P:(t + 1) * P, oc * RC:(oc + 1) * RC],
                        in_=ev)

            # ---------------- emission schedule ----------------
            # preamble: deadline-ordered loads + head-pair-0 projections
            load_col_bias(bq_sb, bq)
            nc.sync.dma_start(out=wq_sb,
                              in_=wqT.ap().rearrange("(k p) n -> p k n", p=P))
            xq0 = qk_load(xqT_v, 0, "xq")
            xq1 = qk_load(xqT_v, 1, "xq")
            qk_mm(0, wq_sb, bq_sb, xq0, qt_sb, 0)
            qk_mm(0, wq_sb, bq_sb, xq1, qt_sb, 1)
            load_col_bias(bk_sb, bk)
            nc.sync.dma_start(out=wk_sb,
                              in_=wkT.ap().rearrange("(k p) n -> p k n", p=P))
            xk0 = qk_load(xkT_v, 0, "xk")
            qk_mm(0, wk_sb, bk_sb, xk0, kt_sb, 0)
            bv_ap = bv.ap()
            nc.sync.dma_start(
                out=bv_bc,
                in_=bass.AP(tensor=bv_ap.tensor, offset=bv_ap.offset,
                            ap=[[0, P]] + bv_ap.ap.copy()))
            nc.sync.dma_start(out=wv_sb,
                              in_=wvT.ap().rearrange("(k p) n -> p k n", p=P))
            nc.vector.memset(v_sb[:, :, :, DK:DK + 1], 1.0)
            vr = {0: v_load(0)}
            load_mask_g2(0)
            vr[1] = v_load(1)
            load_mask_g2(1)
            vr[2] = v_load(2)

            # units for attn(0,0): K/Q chunk loads+mms, masks, V pipeline
            hold = {}
            u00 = {st: [] for st in range(ST)}

            def addu(st, fn):
                u00[st].append(fn)

            addu(0, lambda: v_mm(0, vr.pop(0)))
            for st in range(ST):
                if st + 3 < ST:
                    addu(st, lambda st=st: vr.__setitem__(st + 3, v_load(st + 3)))
                if st + 1 < ST:
                    addu(st, lambda st=st: v_mm(st + 1, vr.pop(st + 1)))
            for j, st in ((1, 0), (2, 4), (3, 8)):
                addu(st, lambda j=j: hold.__setitem__(
                    ("xk", j), qk_load(xkT_v, j, "xk")))
                addu(st + 2, lambda j=j: qk_mm(
                    0, wk_sb, bk_sb, hold.pop(("xk", j)), kt_sb, j))
            for g2 in range(2, 8):
                addu((g2 - 2) * 2, lambda g2=g2: load_mask_g2(g2))
            for j, st in ((2, 10), (3, 11)):
                addu(st, lambda j=j: hold.__setitem__(
                    ("xq", j), qk_load(xqT_v, j, "xq")))
                addu(st + 2, lambda j=j: qk_mm(
                    0, wq_sb, bq_sb, hold.pop(("xq", j)), qt_sb, j))

            u01 = {0: [lambda: nc.sync.dma_start(
                out=wo_sb, in_=woT.ap().rearrange("(k p) n -> p k n", p=P))]}

            # background units: head-pairs 1-3 projections, loads 2 ahead
            proj_descs = []
            for hp in range(1, NHP):
                for tc_ in range(4):
                    proj_descs.append((hp, wq_sb, bq_sb, xqT_v, qt_sb, tc_, "xq"))
                for tc_ in range(4):
                    proj_descs.append((hp, wk_sb, bk_sb, xkT_v, kt_sb, tc_, "xk"))
            pend = {}

            def bg_slot(i):
                def run():
                    if i < len(proj_descs):
                        _, _, _, xv_, _, tc_, tag = proj_descs[i]
                        pend[i] = qk_load(xv_, tc_, tag)
                    j = i - 2
                    if 0 <= j < len(proj_descs):
                        hp, w, b_, _, dst, tc_, _ = proj_descs[j]
                        qk_mm(hp, w, b_, pend.pop(j), dst, tc_)
                return run

            bg = [bg_slot(i) for i in range(len(proj_descs) + 2)]
            bg.extend([lambda: None] * 100)

            o_units = {}
            for t in range(RT // 2):
                o_units[2 * t] = [lambda t=t: o_tp(t, scpool, nc.vector.tensor_copy)]
                o_units[2 * t + 1] = [lambda t=t: o_mm(t, scpool, nc.vector.tensor_copy)]

            rhalves = [
                (0, 0, u00, None, ()),
                (0, 1, u01, None, ()),
                (1, 0, None, bg, (1, 3, 5, 7, 9, 11, 13)),
                (1, 1, None, bg, (1, 3, 5, 7, 9, 11, 13)),
                (2, 0, None, bg, (1, 3, 5, 7, 9, 11, 13)),
                (2, 1, None, bg, (1, 3, 5, 7, 9, 11, 13)),
                (3, 0, None, None, ()),
                (3, 1, None, None, ()),
                (4, 0, None, None, ()),
                (4, 1, None, None, ()),
                (5, 0, None, None, ()),
                (5, 1, None, None, ()),
                (6, 0, None, None, ()),
                (6, 1, None, None, ()),
                (7, 0, None, None, ()),
                (7, 1, o_units, None, ()),
            ]
            emit_sc(0, 0, 0)
            emit_sc(0, 0, 1)
            for idx in range(len(rhalves)):
                if idx == 8:
                    w_scope.__exit__(None, None, None)
                    o_scope = tc.tile_pool(name="ostage", bufs=2)
                    ostage = o_scope.__enter__()
                attn_rhalf_body(idx, rhalves)
            ps_scope_xt.__exit__(None, None, None)
            ps_scope_sc.__exit__(None, None, None)
            ps_scope_o = tc.tile_pool(name="opj", bufs=5, space="PSUM")
            opj = ps_scope_o.__enter__()
            o_tp(RT // 2, opj, nc.scalar.copy)
            for t in range(RT // 2, RT):
                if t + 1 < RT:
                    o_tp(t + 1, opj, nc.scalar.copy)
                o_mm(t, opj, nc.scalar.copy)
            ps_scope_o.__exit__(None, None, None)
            o_scope.__exit__(None, None, None)
    nc.finalize()
    return nc


_NC_CACHE = {}


def _get_nc():
    if "nc" not in _NC_CACHE:
        _NC_CACHE["nc"] = build_nc()
    return _NC_CACHE["nc"]


def make_in_maps(query, key, value, mask, Wq, bq, Wk, bk, Wv, bv, Wo, bo):
    import ml_dtypes
    bf16 = ml_dtypes.bfloat16

    query = np.asarray(query, np.float32)
    key = np.asarray(key, np.float32)
    value = np.asarray(value, np.float32)
    mask = np.asarray(mask)

    xT = {}
    mT = {}
    for b in range(B):
        xT[("q", b)] = np.ascontiguousarray(query[b].T.astype(bf16))
        xT[("k", b)] = np.ascontiguousarray(key[b].T.astype(bf16))
        xT[("v", b)] = np.ascontiguousarray(value[b].T.astype(bf16))
        mT[b] = np.ascontiguousarray(mask[b].T.astype(np.float32).astype(bf16))

    wg = {}
    for g in range(2):
        sl = slice(g * DH, (g + 1) * DH)
        wg[("q", g)] = np.ascontiguousarray(
            np.asarray(Wq, np.float32)[sl, :].T.astype(bf16))
        wg[("k", g)] = np.ascontiguousarray(
            np.asarray(Wk, np.float32)[sl, :].T.astype(bf16))
        wg[("v", g)] = np.ascontiguousarray(
            np.asarray(Wv, np.float32)[sl, :].T.astype(bf16))
        wg[("o", g)] = np.ascontiguousarray(
            np.asarray(Wo, np.float32)[:, sl].T.astype(bf16))
        wg[("bq", g)] = np.ascontiguousarray(np.asarray(bq, np.float32)[sl])
        wg[("bk", g)] = np.ascontiguousarray(np.asarray(bk, np.float32)[sl])
        wg[("bv", g)] = np.ascontiguousarray(np.asarray(bv, np.float32)[sl])

    in_maps = []
    for c in range(NCORES):
        b, g = c // 2, c % 2
        in_maps.append({
            "xqT": xT[("q", b)], "xkT": xT[("k", b)], "xvT": xT[("v", b)],
            "wqT": wg[("q", g)], "wkT": wg[("k", g)], "wvT": wg[("v", g)],
            "woT": wg[("o", g)],
            "bq": wg[("bq", g)], "bk": wg[("bk", g)], "bv": wg[("bv", g)],
            "mskT": mT[b],
        })
    return in_maps


def kernel(query, key, value, mask, Wq, bq, Wk, bk, Wv, bv, Wo, bo):
    from concourse.bass_utils import run_bass_kernel_spmd

    nc = _get_nc()
    in_maps = make_in_maps(query, key, value, mask,
                           Wq, bq, Wk, bk, Wv, bv, Wo, bo)
    res = run_bass_kernel_spmd(nc, in_maps, list(range(NCORES)))
    bo32 = np.asarray(bo, np.float32)
    full = np.empty((B, S, D), dtype=np.float32)
    for b in range(B):
        full[b] = (res.results[2 * b]["out"]
                   + res.results[2 * b + 1]["out"] + bo32)
    return full
